# revision 1
# baseline (speedup 1.0000x reference)
"""EdgeConv GNN message-passing kernel for 8 Trainium2 NeuronCores.

Math refactor of the reference:
    e = [x_i, x_j - x_i]; h = relu(e@W1 + b1)@W2 + b2; agg = segment_mean(h, dst)
    out = relu(relu(agg)@W3 + b3)... (see reference)
is rewritten as
    u = x @ (W1a - W1b) + b1        (per node, W1a = W1[:128], W1b = W1[128:])
    v = x @ W1b                      (per node)
    m_i = mean_{e: dst=i} relu(u_i + v_{src(e)})
    agg_i = m_i @ W2 + b2            (for deg>0; isolated nodes fixed on host)
    out = relu(relu(agg) @ W3 + b3) @ W4 + b4
so the per-edge work is two row fetches + add + relu, and all matmuls act on
node-sized tensors.

Device mapping (one SPMD program, 8 cores, no collectives):
 - nodes sorted by (deg_lo, deg_hi) and dealt round-robin into
   49 positions x 8 cores x 128 lanes; each core owns 6272 node slots.
 - phase A: each core computes u (own nodes) and the full v table, written
   to two DRAM windows v_lo/v_hi (each <= 32768 rows, for int16 gather idx).
 - phase B: per position, two dma_gathers fill a slot grid
   z[128 lanes, K, 128] with v[src]; padding slots gather a -1e30 row.
   DVE adds u (free-dim broadcast) in place, ACT applies relu, DVE reduces
   over slots, then the trailing MLP runs on PE/ACT in transposed layout.
"""
import sys

sys.path.insert(0, "/opt/trn_rl_repo")

import numpy as np

N = 50000
F = 128
P = 128
N_CORES = 8
N_PAD = 50048            # 391 * 128, node count padded for 128-tiles
RANKN = 49 * 1024        # 50176 rank slots (49 positions x 8 cores x 128 lanes)
NPOS = 49
OWN = NPOS * P           # 6272 node slots per core
SPLIT = 32767            # src <= 32766 -> lo window, src >= 32767 -> hi window
V_LO = 32768             # row 0 = NEG pad, rows 1..32767 = v[0..32766]
V_HI = (N_PAD - SPLIT) + 1   # rows 0..17280 = v[32767..50047], last row = NEG pad
NEG = -1.0e30

_compiled_cache = {}
PROFILE = False
LAST_EXEC_NS = None
LAST_TRACE = None
LAST_TMPDIR = None


def _preprocess(x, src, dst):
    """Host-side integer/index preprocessing. Returns per-core tensors and
    the static config that shapes the device program."""
    E = src.shape[0]
    A_mask = src <= SPLIT - 1

    cnt = np.bincount(dst, minlength=N_PAD).astype(np.int64)
    cntA = np.bincount(dst[A_mask], minlength=N_PAD).astype(np.int64)
    cntB = cnt - cntA

    # sort nodes by (deg_lo desc, deg_hi desc): per-position max degree ~ mean
    order = np.lexsort((-cntB, -cntA))           # [N_PAD] old ids by rank
    old_of_rank = np.full(RANKN, -1, dtype=np.int64)
    old_of_rank[:N_PAD] = order
    rank_of_old = np.empty(N_PAD, dtype=np.int64)
    rank_of_old[order] = np.arange(N_PAD)

    cntA_r = np.zeros(RANKN, dtype=np.int64)
    cntB_r = np.zeros(RANKN, dtype=np.int64)
    cntA_r[:N_PAD] = cntA[order]
    cntB_r[:N_PAD] = cntB[order]
    K_A = cntA_r.reshape(NPOS, 1024).max(axis=1)          # [49]
    K_B = cntB_r.reshape(NPOS, 1024).max(axis=1)
    baseA = np.concatenate([[0], np.cumsum(K_A)])          # [50]
    baseB = np.concatenate([[0], np.cumsum(K_B)])
    totA, totB = int(baseA[-1]), int(baseB[-1])

    # place each edge into its (core, half, flat slot)
    r_dst = rank_of_old[dst]
    half = (~A_mask).astype(np.int64)
    eorder = np.lexsort((src, half, r_dst))
    rs = r_dst[eorder]
    hs = half[eorder]
    ss = src[eorder]
    grp = rs * 2 + hs
    newg = np.r_[True, np.diff(grp) != 0]
    gid = np.cumsum(newg) - 1
    first = np.flatnonzero(newg)
    j = np.arange(E) - first[gid]                # slot index within (node, half)

    p = rs // 1024
    w = rs % 1024
    core = w // P
    lane = w % P

    idxA = [np.zeros(totA * P, dtype=np.int16) for _ in range(N_CORES)]
    idxB = [np.full(totB * P, V_HI - 1, dtype=np.int16) for _ in range(N_CORES)]
    flatA = baseA[p] * P + j * P + lane
    flatB = baseB[p] * P + j * P + lane
    valA = (ss + 1).astype(np.int16)
    valB = (ss - SPLIT).astype(np.int16)
    for c in range(N_CORES):
        mA = (core == c) & (hs == 0)
        mB = (core == c) & (hs == 1)
        idxA[c][flatA[mA]] = valA[mA]
        idxB[c][flatB[mB]] = valB[mB]

    def wrap(flat):
        # dma_gather idx layout: [16, n/16] with elem i at [i%16, i//16],
        # replicated across the 8 gpsimd cores -> [128, n/16]
        if flat.size == 0:
            return np.zeros((P, 0), dtype=np.int16)
        wr = flat.reshape(-1, 16).T.copy()
        return np.tile(wr, (8, 1))

    idxA_t = [wrap(a) for a in idxA]
    idxB_t = [wrap(b) for b in idxB]

    # per-core recip / xT slices / output mapping
    xT = np.zeros((F, N_PAD), dtype=np.float32)
    xT[:, :N] = x.T
    recip_all = (1.0 / np.maximum(cnt, 1)).astype(np.float32)

    xT_own, recip_own, ranks_core = [], [], []
    for c in range(N_CORES):
        ranks = (np.arange(NPOS)[:, None] * 1024 + c * P + np.arange(P)[None, :])
        ranks = ranks.reshape(-1)                # [6272] rank per (p, lane)
        olds = old_of_rank[ranks]
        xo = np.zeros((F, OWN), dtype=np.float32)
        valid = olds >= 0
        xo[:, valid] = xT[:, olds[valid]]
        xT_own.append(np.ascontiguousarray(xo))
        rc = np.zeros((P, NPOS), dtype=np.float32)
        rc_flat = np.zeros(OWN, dtype=np.float32)
        rc_flat[valid] = recip_all[olds[valid]]
        # rc[lane, p] = recip of rank (p, lane)
        rc[:, :] = rc_flat.reshape(NPOS, P).T
        recip_own.append(np.ascontiguousarray(rc))
        ranks_core.append(olds)

    cfg = (tuple(int(k) for k in K_A), tuple(int(k) for k in K_B))
    return dict(
        xT=np.ascontiguousarray(xT), xT_own=xT_own, recip_own=recip_own,
        idxA=idxA_t, idxB=idxB_t, K_A=K_A, K_B=K_B, baseA=baseA, baseB=baseB,
        ranks_core=ranks_core, cnt=cnt, cfg=cfg,
    )


def _build_program(cfg):
    """Build + compile the SPMD bass program for the given (K_A, K_B)."""
    import concourse.bass as bass
    import concourse.bacc as bacc
    import concourse.mybir as mybir
    import concourse.tile as tile
    from concourse.masks import make_identity

    K_A, K_B = cfg
    f32 = mybir.dt.float32
    i16 = mybir.dt.int16
    totA = sum(K_A)
    totB = sum(K_B)
    baseA = np.concatenate([[0], np.cumsum(K_A)]).astype(int)
    baseB = np.concatenate([[0], np.cumsum(K_B)]).astype(int)

    nc = bacc.Bacc("TRN2", target_bir_lowering=False, debug=False,
                   num_devices=N_CORES, num_swdge_queues=4)

    xT_d = nc.dram_tensor("xT", [F, N_PAD], f32, kind="ExternalInput")
    xTo_d = nc.dram_tensor("xT_own", [F, OWN], f32, kind="ExternalInput")
    Wd_d = nc.dram_tensor("Wd", [F, F], f32, kind="ExternalInput")
    W1b_d = nc.dram_tensor("W1b", [F, F], f32, kind="ExternalInput")
    W2_d = nc.dram_tensor("W2", [F, F], f32, kind="ExternalInput")
    W3_d = nc.dram_tensor("W3", [F, 64], f32, kind="ExternalInput")
    W4_d = nc.dram_tensor("W4", [64, F], f32, kind="ExternalInput")
    b1b_d = nc.dram_tensor("b1_bcast", [P, F], f32, kind="ExternalInput")
    b2c_d = nc.dram_tensor("b2col", [F, 1], f32, kind="ExternalInput")
    b3c_d = nc.dram_tensor("b3col", [64, 1], f32, kind="ExternalInput")
    b4c_d = nc.dram_tensor("b4col", [F, 1], f32, kind="ExternalInput")
    recip_d = nc.dram_tensor("recip", [P, NPOS], f32, kind="ExternalInput")
    idxA_d = nc.dram_tensor("idxA", [P, max(totA * 8, 1)], i16,
                            kind="ExternalInput")
    idxB_d = nc.dram_tensor("idxB", [P, max(totB * 8, 1)], i16,
                            kind="ExternalInput")

    v_lo = nc.dram_tensor("v_lo", [V_LO, F], f32)
    v_hi = nc.dram_tensor("v_hi", [V_HI, F], f32)
    out_d = nc.dram_tensor("out", [OWN, F], f32, kind="ExternalOutput")

    with tile.TileContext(nc) as tc:
        with (
            tc.tile_pool(name="persist", bufs=1) as pers,
            tc.tile_pool(name="stage", bufs=4) as stage,
            tc.tile_pool(name="zpool", bufs=12) as zpool,
            tc.tile_pool(name="small", bufs=3) as small,
            tc.tile_pool(name="psA", bufs=4, space="PSUM") as psA,
            tc.tile_pool(name="psT", bufs=4, space="PSUM") as psT,
        ):
            # ---- constants ----
            Wd_t = pers.tile([F, F], f32)
            nc.sync.dma_start(out=Wd_t[:], in_=Wd_d[:])
            W1b_t = pers.tile([F, F], f32)
            nc.sync.dma_start(out=W1b_t[:], in_=W1b_d[:])
            W2_t = pers.tile([F, F], f32)
            nc.sync.dma_start(out=W2_t[:], in_=W2_d[:])
            W3_t = pers.tile([F, 64], f32)
            nc.sync.dma_start(out=W3_t[:], in_=W3_d[:])
            W4_t = pers.tile([64, F], f32)
            nc.sync.dma_start(out=W4_t[:], in_=W4_d[:])
            b1b_t = pers.tile([P, F], f32)
            nc.sync.dma_start(out=b1b_t[:], in_=b1b_d[:])
            b2c_t = pers.tile([F, 1], f32)
            nc.sync.dma_start(out=b2c_t[:], in_=b2c_d[:])
            b3c_t = pers.tile([64, 1], f32)
            nc.sync.dma_start(out=b3c_t[:], in_=b3c_d[:])
            b4c_t = pers.tile([F, 1], f32)
            nc.sync.dma_start(out=b4c_t[:], in_=b4c_d[:])
            recip_t = pers.tile([P, NPOS], f32)
            nc.sync.dma_start(out=recip_t[:], in_=recip_d[:])
            ident = pers.tile([P, P], f32)
            make_identity(nc, ident[:])
            u_t = pers.tile([P, OWN], f32)      # all own u rows stay in SBUF
            if totA:
                idxA_t = pers.tile([P, totA * 8], i16)
                nc.sync.dma_start(out=idxA_t[:], in_=idxA_d[:])
            if totB:
                idxB_t = pers.tile([P, totB * 8], i16)
                nc.sync.dma_start(out=idxB_t[:], in_=idxB_d[:])

            neg_t = pers.tile([1, F], f32)
            nc.vector.memset(neg_t[:], NEG)
            nc.sync.dma_start(out=v_lo[0:1, :], in_=neg_t[:])
            nc.sync.dma_start(out=v_hi[V_HI - 1:V_HI, :], in_=neg_t[:])

            # ---- phase A: u (own nodes) ----
            for p in range(NPOS):
                up = psA.tile([P, F], f32, tag="pa")
                xo = stage.tile([F, P], f32, tag="xo")
                nc.sync.dma_start(out=xo[:], in_=xTo_d[:, p * P:(p + 1) * P])
                nc.tensor.matmul(out=up[:], lhsT=xo[:], rhs=Wd_t[:],
                                 start=True, stop=True)
                nc.vector.tensor_tensor(out=u_t[:, p * P:(p + 1) * P],
                                        in0=up[:], in1=b1b_t[:],
                                        op=mybir.AluOpType.add)

            # ---- phase A: v table (all nodes), v_lo first ----
            NT = N_PAD // P
            BT = 4                      # tiles per write batch
            for b0 in range(0, NT, BT):
                bt = min(BT, NT - b0)
                bn0 = b0 * P
                xs = stage.tile([F, P * bt], f32, tag="xs")
                nc.sync.dma_start(out=xs[:], in_=xT_d[:, bn0:bn0 + P * bt])
                vstage = stage.tile([P, bt, F], f32, tag="vs")
                for ti in range(bt):
                    vp = psA.tile([P, F], f32, tag="pa")
                    nc.tensor.matmul(out=vp[:],
                                     lhsT=xs[:, ti * P:(ti + 1) * P],
                                     rhs=W1b_t[:], start=True, stop=True)
                    if ti % 2 == 0:
                        nc.vector.tensor_copy(out=vstage[:, ti, :], in_=vp[:])
                    else:
                        nc.scalar.activation(vstage[:, ti, :], vp[:],
                                             mybir.ActivationFunctionType.Copy)
                lo_cnt = max(0, min(P * bt, SPLIT - bn0))
                if lo_cnt == P * bt and bt == BT:
                    # whole batch below the split: one strided DMA
                    nc.sync.dma_start(
                        out=v_lo[bn0 + 1:bn0 + 1 + P * bt, :]
                            .rearrange("(t q) f -> q t f", q=P),
                        in_=vstage[:])
                elif lo_cnt == 0 and bt == BT:
                    h0 = bn0 - SPLIT
                    nc.sync.dma_start(
                        out=v_hi[h0:h0 + P * bt, :]
                            .rearrange("(t q) f -> q t f", q=P),
                        in_=vstage[:])
                else:
                    # boundary / tail batch: per-tile DMAs
                    for ti in range(bt):
                        n0 = bn0 + ti * P
                        lc = max(0, min(P, SPLIT - n0))
                        if lc > 0:
                            nc.sync.dma_start(
                                out=v_lo[n0 + 1:n0 + 1 + lc, :],
                                in_=vstage[:lc, ti, :])
                        if lc < P:
                            h0 = n0 + lc - SPLIT
                            nc.sync.dma_start(
                                out=v_hi[h0:h0 + (P - lc), :],
                                in_=vstage[lc:, ti, :])

            _q = [0]
            # ---- phase B: per position gather + add + relu + reduce + MLP ----
            for p in range(NPOS):
                kA, kB = int(K_A[p]), int(K_B[p])
                K = kA + kB
                if K == 0:
                    continue
                CH = 8   # 1024 idxs per gather = single_packet limit
                chunks = ([("A", c0, min(CH, kA - c0)) for c0 in range(0, kA, CH)]
                          + [("B", c0, min(CH, kB - c0)) for c0 in range(0, kB, CH)])
                s_t = small.tile([P, F], f32, tag="s")
                for ci, (half, c0, cw) in enumerate(chunks):
                    z = zpool.tile([P, CH, F], f32, tag="z")
                    if half == "A":
                        nc.gpsimd.dma_gather(
                            out_ap=z[:, :cw, :], in_ap=v_lo[:],
                            idxs_ap=idxA_t[:, (baseA[p] + c0) * 8:
                                           (baseA[p] + c0 + cw) * 8],
                            num_idxs=cw * P, num_idxs_reg=cw * P, elem_size=F,
                            queue_num=_q[0] % 4)
                    else:
                        nc.gpsimd.dma_gather(
                            out_ap=z[:, :cw, :], in_ap=v_hi[:],
                            idxs_ap=idxB_t[:, (baseB[p] + c0) * 8:
                                           (baseB[p] + c0 + cw) * 8],
                            num_idxs=cw * P, num_idxs_reg=cw * P, elem_size=F,
                            queue_num=_q[0] % 4)
                    _q[0] += 1
                    u_b = (u_t[:, p * P:(p + 1) * P]
                           .rearrange("q (k f) -> q k f", k=1)
                           .to_broadcast([P, cw, F]))
                    nc.vector.tensor_tensor(out=z[:, :cw, :], in0=z[:, :cw, :],
                                            in1=u_b, op=mybir.AluOpType.add)
                    nc.scalar.activation(z[:, :cw, :], z[:, :cw, :],
                                         mybir.ActivationFunctionType.Relu)
                    # contiguous halving tree-sum over the slot dim (2x-mode
                    # friendly, vs ~4x slower strided tensor_reduce)
                    w = cw
                    while w > 1:
                        h = w // 2
                        nc.vector.tensor_tensor(
                            out=z[:, :h, :], in0=z[:, :h, :],
                            in1=z[:, w - h:w, :], op=mybir.AluOpType.add)
                        w = w - h
                    if ci == 0:
                        nc.vector.tensor_copy(out=s_t[:], in_=z[:, 0, :])
                    else:
                        nc.vector.tensor_tensor(out=s_t[:], in0=s_t[:],
                                                in1=z[:, 0, :],
                                                op=mybir.AluOpType.add)
                m_t = small.tile([P, F], f32, tag="m")
                nc.scalar.activation(m_t[:], s_t[:],
                                     mybir.ActivationFunctionType.Copy,
                                     scale=recip_t[:, p:p + 1])
                mT_p = psT.tile([P, P], f32, tag="pt")
                nc.tensor.transpose(out=mT_p[:], in_=m_t[:], identity=ident[:])
                mT_t = small.tile([P, P], f32, tag="mTs")
                nc.vector.tensor_copy(out=mT_t[:], in_=mT_p[:])
                o1 = psT.tile([P, P], f32, tag="pt")
                nc.tensor.matmul(out=o1[:], lhsT=W2_t[:], rhs=mT_t[:],
                                 start=True, stop=True)
                h1 = small.tile([F, P], f32, tag="h1")
                nc.scalar.activation(h1[:], o1[:],
                                     mybir.ActivationFunctionType.Relu,
                                     bias=b2c_t[:, :1])
                o2 = psT.tile([64, P], f32, tag="pt")
                nc.tensor.matmul(out=o2[:], lhsT=W3_t[:], rhs=h1[:],
                                 start=True, stop=True)
                h2 = small.tile([64, P], f32, tag="h2")
                nc.scalar.activation(h2[:], o2[:],
                                     mybir.ActivationFunctionType.Relu,
                                     bias=b3c_t[:, :1])
                o3 = psT.tile([P, P], f32, tag="pt")
                nc.tensor.matmul(out=o3[:], lhsT=W4_t[:], rhs=h2[:],
                                 start=True, stop=True)
                yT = small.tile([F, P], f32, tag="yT")
                nc.scalar.activation(yT[:], o3[:],
                                     mybir.ActivationFunctionType.Identity,
                                     bias=b4c_t[:, :1])
                yp = psT.tile([P, P], f32, tag="pt")
                nc.tensor.transpose(out=yp[:], in_=yT[:], identity=ident[:])
                y_t = small.tile([P, F], f32, tag="y")
                nc.vector.tensor_copy(out=y_t[:], in_=yp[:])
                nc.sync.dma_start(out=out_d[p * P:(p + 1) * P, :], in_=y_t[:])

    nc.compile()
    return nc


def _install_ntff_hook():
    """The agent image's antenv lacks axon_hooks; synthesize it so
    run_bass_kernel_spmd(trace=True) can NTFF-profile via libaxon."""
    import types
    if "antenv.axon_hooks" in sys.modules:
        return
    try:
        mod = types.ModuleType("antenv.axon_hooks")
        holder = {}
        mod.set_axon_ntff_profile_hook = lambda h: holder.__setitem__("h", h)
        mod.get_axon_ntff_profile_hook = lambda: holder.get("h")
        import antenv
        sys.modules["antenv.axon_hooks"] = mod
        antenv.axon_hooks = mod
        from trn_agent_boot.trn_boot import _ntff_profile_via_ctypes
        mod.set_axon_ntff_profile_hook(
            _ntff_profile_via_ctypes("/opt/axon/libaxon_pjrt.so"))
    except Exception:
        pass


def kernel(x, edge_index, W1, b1, W2, b2, W3, b3, W4, b4):
    from concourse.bass_utils import run_bass_kernel_spmd

    x = np.asarray(x, dtype=np.float32)
    edge_index = np.asarray(edge_index)
    src = np.asarray(edge_index[0], dtype=np.int64)
    dst = np.asarray(edge_index[1], dtype=np.int64)
    W1 = np.asarray(W1, dtype=np.float32)
    b1 = np.asarray(b1, dtype=np.float32)
    W2 = np.asarray(W2, dtype=np.float32)
    b2 = np.asarray(b2, dtype=np.float32)
    W3 = np.asarray(W3, dtype=np.float32)
    b3 = np.asarray(b3, dtype=np.float32)
    W4 = np.asarray(W4, dtype=np.float32)
    b4 = np.asarray(b4, dtype=np.float32)

    pre = _preprocess(x, src, dst)
    cfg = pre["cfg"]
    if cfg not in _compiled_cache:
        _compiled_cache.clear()
        _compiled_cache[cfg] = _build_program(cfg)
    nc = _compiled_cache[cfg]

    W1a = W1[:F]
    W1b = W1[F:]
    Wd = np.ascontiguousarray(W1a - W1b)
    b1_bcast = np.tile(b1[None, :], (P, 1)).astype(np.float32)

    in_maps = []
    for c in range(N_CORES):
        in_maps.append({
            "xT": pre["xT"], "xT_own": pre["xT_own"][c],
            "Wd": Wd, "W1b": np.ascontiguousarray(W1b),
            "W2": W2, "W3": W3, "W4": W4,
            "b1_bcast": b1_bcast,
            "b2col": np.ascontiguousarray(b2[:, None]),
            "b3col": np.ascontiguousarray(b3[:, None]),
            "b4col": np.ascontiguousarray(b4[:, None]),
            "recip": pre["recip_own"][c],
            "idxA": pre["idxA"][c] if pre["idxA"][c].shape[1] else
                    np.zeros((P, 1), dtype=np.int16),
            "idxB": pre["idxB"][c] if pre["idxB"][c].shape[1] else
                    np.zeros((P, 1), dtype=np.int16),
        })

    kw = {}
    if PROFILE:
        import tempfile
        _install_ntff_hook()
        kw = dict(trace=True, tmpdir=tempfile.mkdtemp(prefix="gnn_trace_"))
    last_err = None
    for attempt in range(3):
        try:
            res = run_bass_kernel_spmd(nc, in_maps, list(range(N_CORES)), **kw)
            break
        except Exception as e:  # device flake: retry
            last_err = e
            import time
            time.sleep(15)
    else:
        raise last_err

    global LAST_EXEC_NS, LAST_TRACE, LAST_TMPDIR
    LAST_EXEC_NS = res.exec_time_ns
    LAST_TRACE = (res.instructions_and_trace[1]
                  if res.instructions_and_trace else None)
    LAST_TMPDIR = kw.get("tmpdir")

    out = np.zeros((N, F), dtype=np.float32)
    for c in range(N_CORES):
        olds = pre["ranks_core"][c]
        valid = (olds >= 0) & (olds < N)
        out[olds[valid]] = res.results[c]["out"][valid]

    # isolated nodes: reference gives agg=0 -> out = relu(b3)@W4 + b4
    iso = np.flatnonzero(pre["cnt"][:N] == 0)
    if iso.size:
        row = np.maximum(b3, 0.0) @ W4 + b4
        out[iso] = row[None, :]
    return out


if __name__ == "__main__":
    sys.path.insert(0, "/root/problem")
    import jax
    import reference
    with jax.default_device(jax.devices("cpu")[0]):
        inputs = reference.setup_inputs()
        expected = np.asarray(reference.reference(**inputs))
    inputs = {k: np.asarray(v) for k, v in inputs.items()}
    got = kernel(**inputs)
    denom = np.abs(expected).max()
    rel = np.abs(got - expected).max() / denom
    print("Relative error:", rel)
    print("PASS" if rel < 2e-2 else "FAIL")



# revision 8
# speedup vs baseline: 1.1617x; 1.1617x over previous
"""EdgeConv GNN message-passing kernel for 8 Trainium2 NeuronCores.

Math refactor of the reference:
    e = [x_i, x_j - x_i]; h = relu(e@W1 + b1)@W2 + b2; agg = segment_mean(h, dst)
    out = relu(relu(agg)@W3 + b3)@W4 + b4
is rewritten as
    u = x @ (W1a - W1b) + b1        (per node, W1a = W1[:128], W1b = W1[128:])
    v = x @ W1b                      (per node)
    m_i = mean_{e: dst=i} relu(u_i + v_{src(e)})
    agg_i = m_i @ W2 + b2            (for deg>0; isolated nodes fixed on host)
    out = relu(relu(agg) @ W3 + b3) @ W4 + b4

Device mapping (one SPMD program, 8 cores, no collectives):
 - nodes sorted by (deg_lo, deg_hi) and dealt round-robin into
   49 positions x 8 cores x 128 lanes; each core owns 6272 node slots.
 - bf16 throughout (inputs quantized on host; 2e-2 rel tol has plenty of
   headroom, ~5e-3 measured): v rows are 256B, DVE/ACT run in 2x mode.
 - v table in DRAM: rows 0..127 = NEG pad, 128..50175 = v[node],
   50176..50303 = NEG pad. Window split at node 17408 so only 136 of the
   391 v tiles must be built before the A-sweep gathers start; the other
   255 are built concurrently with the A sweep.
 - phase B sweeps the lo window (A slots) then the hi window (B slots)
   with GLOBAL 8-slot/1024-idx gather chunks spanning position
   boundaries (1024 idxs = HW limit per dma_gather). Per chunk: DVE adds
   u (free-dim broadcast) per position segment, ACT relu once, DVE
   halving tree-sum per segment into sA.
 - trailing MLP batched 4 positions at a time: ACT recip-scale, PE
   transpose to [f, node], 512-wide matmuls, output written transposed
   [F, OWN] and fixed up on host.
"""
import sys

sys.path.insert(0, "/opt/trn_rl_repo")

import numpy as np
import ml_dtypes

BF16 = ml_dtypes.bfloat16

N = 50000
F = 128
P = 128
N_CORES = 8
N_PAD = 50048            # 391 * 128, node count padded for 128-tiles
RANKN = 49 * 1024        # 50176 rank slots (49 positions x 8 cores x 128 lanes)
NPOS = 49
OWN = NPOS * P           # 6272 node slots per core
SPLIT = 17408            # src < SPLIT -> lo window, else hi window
V_ROWS = N_PAD + 2 * P   # 128 NEG + nodes + 128 NEG
LO_ROWS = SPLIT + P      # lo window rows [0, 17536)
PAD_LO = 0
PAD_HI = LO_ROWS + N_PAD - SPLIT - LO_ROWS + P * 0  # placeholder, set below
PAD_HI = V_ROWS - LO_ROWS - P  # 32640: first NEG row in hi-window coords
NEG = -1.0e30
CH = 8                   # slots per gather chunk (1024 idxs = HW limit)

_compiled_cache = {}
PROFILE = False
LAST_EXEC_NS = None
LAST_TRACE = None
LAST_TMPDIR = None


def _preprocess(x, src, dst):
    """Host-side integer/index preprocessing. Returns per-core tensors and
    the static config that shapes the device program."""
    E = src.shape[0]
    A_mask = src < SPLIT

    cnt = np.bincount(dst, minlength=N_PAD).astype(np.int64)
    cntA = np.bincount(dst[A_mask], minlength=N_PAD).astype(np.int64)
    cntB = cnt - cntA

    # sort nodes by (deg_lo desc, deg_hi desc): per-position max degree ~ mean
    order = np.lexsort((-cntB, -cntA))           # [N_PAD] old ids by rank
    old_of_rank = np.full(RANKN, -1, dtype=np.int64)
    old_of_rank[:N_PAD] = order
    rank_of_old = np.empty(N_PAD, dtype=np.int64)
    rank_of_old[order] = np.arange(N_PAD)

    cntA_r = np.zeros(RANKN, dtype=np.int64)
    cntB_r = np.zeros(RANKN, dtype=np.int64)
    cntA_r[:N_PAD] = cntA[order]
    cntB_r[:N_PAD] = cntB[order]
    K_A = cntA_r.reshape(NPOS, 1024).max(axis=1)          # [49]
    K_B = cntB_r.reshape(NPOS, 1024).max(axis=1)
    baseA = np.concatenate([[0], np.cumsum(K_A)])          # [50]
    baseB = np.concatenate([[0], np.cumsum(K_B)])
    totA, totB = int(baseA[-1]), int(baseB[-1])

    # place each edge into its (core, half, flat slot)
    r_dst = rank_of_old[dst]
    half = (~A_mask).astype(np.int64)
    eorder = np.lexsort((src, half, r_dst))
    rs = r_dst[eorder]
    hs = half[eorder]
    ss = src[eorder]
    grp = rs * 2 + hs
    newg = np.r_[True, np.diff(grp) != 0]
    gid = np.cumsum(newg) - 1
    first = np.flatnonzero(newg)
    j = np.arange(E) - first[gid]                # slot index within (node, half)

    p = rs // 1024
    w = rs % 1024
    core = w // P
    lane = w % P

    idxA = [np.full(totA * P, PAD_LO, dtype=np.int16) for _ in range(N_CORES)]
    idxB = [np.full(totB * P, PAD_HI, dtype=np.int16) for _ in range(N_CORES)]
    flatA = baseA[p] * P + j * P + lane
    flatB = baseB[p] * P + j * P + lane
    valA = (ss + P).astype(np.int16)             # lo row = node + 128
    valB = (ss - SPLIT).astype(np.int16)         # hi-window row
    for c in range(N_CORES):
        mA = (core == c) & (hs == 0)
        mB = (core == c) & (hs == 1)
        idxA[c][flatA[mA]] = valA[mA]
        idxB[c][flatB[mB]] = valB[mB]

    def wrap(flat):
        # dma_gather idx layout: [16, n/16] with elem i at [i%16, i//16],
        # replicated across the 8 gpsimd cores -> [128, n/16]
        if flat.size == 0:
            return np.zeros((P, 1), dtype=np.int16)
        wr = flat.reshape(-1, 16).T.copy()
        return np.tile(wr, (8, 1))

    idxA_t = [wrap(a) for a in idxA]
    idxB_t = [wrap(b) for b in idxB]

    # transposed node features (bf16) + per-core own slices
    xT = np.zeros((F, N_PAD), dtype=BF16)
    xT[:, :N] = x.T.astype(BF16)
    recip_all = (1.0 / np.maximum(cnt, 1)).astype(np.float32)

    xT_own, recip_own, ranks_core = [], [], []
    for c in range(N_CORES):
        ranks = (np.arange(NPOS)[:, None] * 1024 + c * P + np.arange(P)[None, :])
        ranks = ranks.reshape(-1)                # [6272] rank per (p, lane)
        olds = old_of_rank[ranks]
        xo = np.zeros((F, OWN), dtype=BF16)
        valid = olds >= 0
        xo[:, valid] = xT[:, olds[valid]]
        xT_own.append(np.ascontiguousarray(xo))
        rc_flat = np.zeros(OWN, dtype=np.float32)
        rc_flat[valid] = recip_all[olds[valid]]
        # rc[lane, p] = recip of rank (p, lane)
        rc = np.ascontiguousarray(rc_flat.reshape(NPOS, P).T)
        recip_own.append(rc)
        ranks_core.append(olds)

    cfg = (tuple(int(k) for k in K_A), tuple(int(k) for k in K_B))
    return dict(
        xT=np.ascontiguousarray(xT), xT_own=xT_own, recip_own=recip_own,
        idxA=idxA_t, idxB=idxB_t, K_A=K_A, K_B=K_B,
        ranks_core=ranks_core, cnt=cnt, cfg=cfg,
    )


def _chunks(tot):
    return [(s0, min(CH, tot - s0)) for s0 in range(0, tot, CH)]


def _segments(base, s0, sw):
    """Positions overlapping chunk slot range [s0, s0+sw) -> (a, b, p) local."""
    segs = []
    for p in range(NPOS):
        a = max(int(base[p]), s0)
        b = min(int(base[p + 1]), s0 + sw)
        if a < b:
            segs.append((a - s0, b - s0, p))
    return segs


def _build_program(cfg):
    """Build + compile the SPMD bass program for the given (K_A, K_B)."""
    import concourse.bass as bass
    import concourse.bacc as bacc
    import concourse.mybir as mybir
    import concourse.tile as tile
    from concourse.masks import make_identity

    K_A, K_B = cfg
    f32 = mybir.dt.float32
    bf16 = mybir.dt.bfloat16
    i16 = mybir.dt.int16
    totA = sum(K_A)
    totB = sum(K_B)
    baseA = np.concatenate([[0], np.cumsum(K_A)]).astype(int)
    baseB = np.concatenate([[0], np.cumsum(K_B)]).astype(int)

    nc = bacc.Bacc("TRN2", target_bir_lowering=False, debug=False,
                   num_devices=N_CORES, num_swdge_queues=4)

    xT_d = nc.dram_tensor("xT", [F, N_PAD], bf16, kind="ExternalInput")
    xTo_d = nc.dram_tensor("xT_own", [F, OWN], bf16, kind="ExternalInput")
    Wd_d = nc.dram_tensor("Wd", [F, F], bf16, kind="ExternalInput")
    W1b_d = nc.dram_tensor("W1b", [F, F], bf16, kind="ExternalInput")
    W2_d = nc.dram_tensor("W2", [F, F], bf16, kind="ExternalInput")
    W3_d = nc.dram_tensor("W3", [F, 64], bf16, kind="ExternalInput")
    W4_d = nc.dram_tensor("W4", [64, F], bf16, kind="ExternalInput")
    b1b_d = nc.dram_tensor("b1_bcast", [P, F], f32, kind="ExternalInput")
    b2c_d = nc.dram_tensor("b2col", [F, 1], f32, kind="ExternalInput")
    b3c_d = nc.dram_tensor("b3col", [64, 1], f32, kind="ExternalInput")
    b4c_d = nc.dram_tensor("b4col", [F, 1], f32, kind="ExternalInput")
    recip_d = nc.dram_tensor("recip", [P, NPOS], f32, kind="ExternalInput")
    idxA_d = nc.dram_tensor("idxA", [P, max(totA * 8, 1)], i16,
                            kind="ExternalInput")
    idxB_d = nc.dram_tensor("idxB", [P, max(totB * 8, 1)], i16,
                            kind="ExternalInput")

    v_d = nc.dram_tensor("v", [V_ROWS, F], bf16)
    out_d = nc.dram_tensor("out", [F, OWN], f32, kind="ExternalOutput")

    chunksA = _chunks(totA)
    chunksB = _chunks(totB)

    with tile.TileContext(nc) as tc:
        with (
            tc.tile_pool(name="persist", bufs=1) as pers,
            tc.tile_pool(name="stage", bufs=4) as stage,
            tc.tile_pool(name="zpool", bufs=12) as zpool,
            tc.tile_pool(name="small", bufs=3) as small,
            tc.tile_pool(name="psV", bufs=2, space="PSUM") as psV,
            tc.tile_pool(name="psT", bufs=2, space="PSUM") as psT,
            tc.tile_pool(name="psG", bufs=2, space="PSUM") as psG,
        ):
            # ---- constants ----
            Wd_t = pers.tile([F, F], bf16)
            nc.sync.dma_start(out=Wd_t[:], in_=Wd_d[:])
            W1b_t = pers.tile([F, F], bf16)
            nc.sync.dma_start(out=W1b_t[:], in_=W1b_d[:])
            W2_t = pers.tile([F, F], bf16)
            nc.sync.dma_start(out=W2_t[:], in_=W2_d[:])
            W3_t = pers.tile([F, 64], bf16)
            nc.sync.dma_start(out=W3_t[:], in_=W3_d[:])
            W4_t = pers.tile([64, F], bf16)
            nc.sync.dma_start(out=W4_t[:], in_=W4_d[:])
            b1b_t = pers.tile([P, F], f32)
            nc.sync.dma_start(out=b1b_t[:], in_=b1b_d[:])
            b2c_t = pers.tile([F, 1], f32)
            nc.sync.dma_start(out=b2c_t[:], in_=b2c_d[:])
            b3c_t = pers.tile([64, 1], f32)
            nc.sync.dma_start(out=b3c_t[:], in_=b3c_d[:])
            b4c_t = pers.tile([F, 1], f32)
            nc.sync.dma_start(out=b4c_t[:], in_=b4c_d[:])
            recip_t = pers.tile([P, NPOS], f32)
            nc.sync.dma_start(out=recip_t[:], in_=recip_d[:])
            idxA_t = pers.tile([P, max(totA * 8, 1)], i16)
            nc.sync.dma_start(out=idxA_t[:], in_=idxA_d[:])
            idxB_t = pers.tile([P, max(totB * 8, 1)], i16)
            nc.sync.dma_start(out=idxB_t[:], in_=idxB_d[:])
            ident = pers.tile([P, P], bf16)
            make_identity(nc, ident[:])

            neg_t = pers.tile([P, F], bf16)
            nc.vector.memset(neg_t[:], NEG)
            nc.sync.dma_start(out=v_d[0:P, :], in_=neg_t[:])
            nc.sync.dma_start(out=v_d[V_ROWS - P:V_ROWS, :], in_=neg_t[:])

            u_t = pers.tile([P, OWN], bf16)     # u[lane, pos*F + f]
            sA_t = pers.tile([P, OWN], bf16)    # s[lane, pos*F + f]
            nc.vector.memset(sA_t[:], 0.0)

            # ---- u = x@Wd + b1 (lane-major: u_t block p = [node, f]) ----
            for p4 in range(0, NPOS, 4):
                pw = min(4, NPOS - p4)
                xo = stage.tile([F, 512], bf16, tag="xo")
                nc.sync.dma_start(out=xo[:, :pw * P],
                                  in_=xTo_d[:, p4 * P:(p4 + pw) * P])
                for i in range(pw):
                    up = psV.tile([P, F], f32, tag="pv")
                    nc.tensor.matmul(out=up[:], lhsT=xo[:, i * P:(i + 1) * P],
                                     rhs=Wd_t[:], start=True, stop=True)
                    nc.vector.tensor_tensor(
                        out=u_t[:, (p4 + i) * P:(p4 + i + 1) * P],
                        in0=up[:], in1=b1b_t[:], op=mybir.AluOpType.add)

            # ---- v tile builder (node tile t -> v rows 128+t*128) ----
            NT = N_PAD // P          # 391 node tiles
            _alt = [0]

            def emit_v_tiles(t0, t1):
                b0 = t0
                while b0 < t1:
                    bt = min(4, t1 - b0)
                    xs = stage.tile([F, 512], bf16, tag="xs")
                    nc.sync.dma_start(out=xs[:, :bt * P],
                                      in_=xT_d[:, b0 * P:(b0 + bt) * P])
                    vstage = stage.tile([P, 4, F], bf16, tag="vs")
                    for ti in range(bt):
                        pv = psV.tile([P, F], f32, tag="pv")
                        nc.tensor.matmul(out=pv[:],
                                         lhsT=xs[:, ti * P:(ti + 1) * P],
                                         rhs=W1b_t[:], start=True, stop=True)
                        if _alt[0] % 2 == 0:
                            nc.vector.tensor_copy(out=vstage[:, ti, :],
                                                  in_=pv[:])
                        else:
                            nc.scalar.activation(
                                vstage[:, ti, :], pv[:],
                                mybir.ActivationFunctionType.Copy)
                        _alt[0] += 1
                    r0 = P + b0 * P
                    nc.sync.dma_start(
                        out=v_d[r0:r0 + bt * P, :]
                            .rearrange("(t q) f -> q t f", q=P),
                        in_=vstage[:, :bt, :])
                    b0 += bt

            LO_T = SPLIT // P        # 136 node tiles in the lo window
            emit_v_tiles(0, LO_T)

            hi_next = [LO_T]

            def emit_hi_share(n_tiles):
                t1 = min(NT, hi_next[0] + n_tiles)
                if t1 > hi_next[0]:
                    emit_v_tiles(hi_next[0], t1)
                    hi_next[0] = t1

            _q = [0]

            def do_chunk(ci, s0, sw, base, idx_t, in_view):
                z = zpool.tile([P, CH, F], bf16, tag="z")
                nc.gpsimd.dma_gather(
                    out_ap=z[:, :sw, :], in_ap=in_view,
                    idxs_ap=idx_t[:, s0 * 8:(s0 + sw) * 8],
                    num_idxs=sw * P, num_idxs_reg=sw * P, elem_size=F,
                    queue_num=_q[0] % 4)
                _q[0] += 1
                segs = _segments(base, s0, sw)
                for (a, b, p) in segs:
                    u_b = (u_t[:, p * P:(p + 1) * P]
                           .rearrange("q (k f) -> q k f", k=1)
                           .to_broadcast([P, b - a, F]))
                    nc.vector.tensor_tensor(out=z[:, a:b, :], in0=z[:, a:b, :],
                                            in1=u_b, op=mybir.AluOpType.add)
                nc.scalar.activation(z[:, :sw, :], z[:, :sw, :],
                                     mybir.ActivationFunctionType.Relu)
                for (a, b, p) in segs:
                    w = b - a
                    while w > 1:
                        h = w // 2
                        nc.vector.tensor_tensor(
                            out=z[:, a:a + h, :], in0=z[:, a:a + h, :],
                            in1=z[:, a + w - h:a + w, :],
                            op=mybir.AluOpType.add)
                        w -= h
                    sl = sA_t[:, p * P:(p + 1) * P]
                    nc.vector.tensor_tensor(out=sl, in0=sl, in1=z[:, a, :],
                                            op=mybir.AluOpType.add)

            def emit_mlp_group(p0, pw):
                mst = small.tile([P, 4, F], bf16, tag="mst")
                for i in range(pw):
                    nc.scalar.activation(mst[:, i, :],
                                         sA_t[:, (p0 + i) * P:(p0 + i + 1) * P],
                                         mybir.ActivationFunctionType.Copy,
                                         scale=recip_t[:, p0 + i:p0 + i + 1])
                mT = small.tile([P, 4 * P], bf16, tag="mT")
                for i in range(pw):
                    pt = psT.tile([P, P], bf16, tag="pt")
                    nc.tensor.transpose(out=pt[:], in_=mst[:, i, :],
                                        identity=ident[:])
                    nc.vector.tensor_copy(out=mT[:, i * P:(i + 1) * P],
                                          in_=pt[:])
                o1 = psG.tile([P, 512], f32, tag="pg")
                nc.tensor.matmul(out=o1[:, :pw * P], lhsT=W2_t[:],
                                 rhs=mT[:, :pw * P], start=True, stop=True)
                h1 = small.tile([F, 512], bf16, tag="h1")
                nc.scalar.activation(h1[:, :pw * P], o1[:, :pw * P],
                                     mybir.ActivationFunctionType.Relu,
                                     bias=b2c_t[:, :1])
                o2 = psG.tile([64, 512], f32, tag="pg2")
                nc.tensor.matmul(out=o2[:, :pw * P], lhsT=W3_t[:],
                                 rhs=h1[:, :pw * P], start=True, stop=True)
                h2 = small.tile([64, 512], bf16, tag="h2")
                nc.scalar.activation(h2[:, :pw * P], o2[:, :pw * P],
                                     mybir.ActivationFunctionType.Relu,
                                     bias=b3c_t[:, :1])
                o3 = psG.tile([P, 512], f32, tag="pg")
                nc.tensor.matmul(out=o3[:, :pw * P], lhsT=W4_t[:],
                                 rhs=h2[:, :pw * P], start=True, stop=True)
                yT = small.tile([F, 512], f32, tag="yT")
                nc.scalar.activation(yT[:, :pw * P], o3[:, :pw * P],
                                     mybir.ActivationFunctionType.Identity,
                                     bias=b4c_t[:, :1])
                nc.sync.dma_start(out=out_d[:, p0 * P:(p0 + pw) * P],
                                  in_=yT[:, :pw * P])

            # ---- A sweep (lo window) with hi v build interleaved ----
            lo_view = v_d[:LO_ROWS, :]
            hi_share = (NT - LO_T) // max(len(chunksA), 1) + 1
            for ci, (s0, sw) in enumerate(chunksA):
                do_chunk(ci, s0, sw, baseA, idxA_t, lo_view)
                emit_hi_share(hi_share)
            emit_hi_share(NT)        # any leftover hi tiles

            # ---- B sweep (hi window) + batched MLP when groups complete ----
            hi_view = v_d[LO_ROWS:, :]
            groups = [(g0, min(4, NPOS - g0)) for g0 in range(0, NPOS, 4)]
            mlp_after = {}           # chunk idx -> [(p0, pw)]
            pre_b = []
            for (g0, gw) in groups:
                lastp = [p for p in range(g0, g0 + gw) if K_B[p] > 0]
                if not lastp:
                    pre_b.append((g0, gw))
                else:
                    ci = (int(baseB[lastp[-1] + 1]) - 1) // CH
                    mlp_after.setdefault(ci, []).append((g0, gw))
            for (g0, gw) in pre_b:
                emit_mlp_group(g0, gw)
            for ci, (s0, sw) in enumerate(chunksB):
                do_chunk(ci, s0, sw, baseB, idxB_t, hi_view)
                for (g0, gw) in mlp_after.get(ci, []):
                    emit_mlp_group(g0, gw)

    nc.compile()
    return nc


def _install_ntff_hook():
    """The agent image's antenv lacks axon_hooks; synthesize it so
    run_bass_kernel_spmd(trace=True) can NTFF-profile via libaxon."""
    import types
    if "antenv.axon_hooks" in sys.modules:
        return
    try:
        mod = types.ModuleType("antenv.axon_hooks")
        holder = {}
        mod.set_axon_ntff_profile_hook = lambda h: holder.__setitem__("h", h)
        mod.get_axon_ntff_profile_hook = lambda: holder.get("h")
        import antenv
        sys.modules["antenv.axon_hooks"] = mod
        antenv.axon_hooks = mod
        from trn_agent_boot.trn_boot import _ntff_profile_via_ctypes
        mod.set_axon_ntff_profile_hook(
            _ntff_profile_via_ctypes("/opt/axon/libaxon_pjrt.so"))
    except Exception:
        pass


def kernel(x, edge_index, W1, b1, W2, b2, W3, b3, W4, b4):
    from concourse.bass_utils import run_bass_kernel_spmd

    x = np.asarray(x, dtype=np.float32)
    edge_index = np.asarray(edge_index)
    src = np.asarray(edge_index[0], dtype=np.int64)
    dst = np.asarray(edge_index[1], dtype=np.int64)
    W1 = np.asarray(W1, dtype=np.float32)
    b1 = np.asarray(b1, dtype=np.float32)
    W2 = np.asarray(W2, dtype=np.float32)
    b2 = np.asarray(b2, dtype=np.float32)
    W3 = np.asarray(W3, dtype=np.float32)
    b3 = np.asarray(b3, dtype=np.float32)
    W4 = np.asarray(W4, dtype=np.float32)
    b4 = np.asarray(b4, dtype=np.float32)

    pre = _preprocess(x, src, dst)
    cfg = pre["cfg"]
    if cfg not in _compiled_cache:
        _compiled_cache.clear()
        _compiled_cache[cfg] = _build_program(cfg)
    nc = _compiled_cache[cfg]

    W1a = W1[:F]
    W1b = W1[F:]
    Wd = np.ascontiguousarray((W1a - W1b).astype(BF16))
    b1_bcast = np.tile(b1[None, :], (P, 1)).astype(np.float32)

    in_maps = []
    for c in range(N_CORES):
        in_maps.append({
            "xT": pre["xT"], "xT_own": pre["xT_own"][c],
            "Wd": Wd, "W1b": np.ascontiguousarray(W1b.astype(BF16)),
            "W2": W2.astype(BF16), "W3": W3.astype(BF16),
            "W4": W4.astype(BF16),
            "b1_bcast": b1_bcast,
            "b2col": np.ascontiguousarray(b2[:, None]),
            "b3col": np.ascontiguousarray(b3[:, None]),
            "b4col": np.ascontiguousarray(b4[:, None]),
            "recip": pre["recip_own"][c],
            "idxA": pre["idxA"][c],
            "idxB": pre["idxB"][c],
        })

    kw = {}
    if PROFILE:
        import tempfile
        _install_ntff_hook()
        kw = dict(trace=True, tmpdir=tempfile.mkdtemp(prefix="gnn_trace_"))
    last_err = None
    for attempt in range(3):
        try:
            res = run_bass_kernel_spmd(nc, in_maps, list(range(N_CORES)), **kw)
            break
        except Exception as e:  # device flake: retry
            last_err = e
            import time
            time.sleep(15)
    else:
        raise last_err

    global LAST_EXEC_NS, LAST_TRACE, LAST_TMPDIR
    LAST_EXEC_NS = res.exec_time_ns
    LAST_TRACE = (res.instructions_and_trace[1]
                  if res.instructions_and_trace else None)
    LAST_TMPDIR = kw.get("tmpdir")

    out = np.zeros((N, F), dtype=np.float32)
    for c in range(N_CORES):
        olds = pre["ranks_core"][c]
        valid = (olds >= 0) & (olds < N)
        out[olds[valid]] = res.results[c]["out"].T[valid]

    # isolated nodes: reference gives agg=0 -> out = relu(b3)@W4 + b4
    iso = np.flatnonzero(pre["cnt"][:N] == 0)
    if iso.size:
        row = np.maximum(b3, 0.0) @ W4 + b4
        out[iso] = row[None, :]
    return out


if __name__ == "__main__":
    sys.path.insert(0, "/root/problem")
    import jax
    import reference
    with jax.default_device(jax.devices("cpu")[0]):
        inputs = reference.setup_inputs()
        expected = np.asarray(reference.reference(**inputs))
    inputs = {k: np.asarray(v) for k, v in inputs.items()}
    got = kernel(**inputs)
    denom = np.abs(expected).max()
    rel = np.abs(got - expected).max() / denom
    print("Relative error:", rel)
    print("PASS" if rel < 2e-2 else "FAIL")


# revision 10
# speedup vs baseline: 1.3361x; 1.1501x over previous
"""EdgeConv GNN message-passing kernel for 8 Trainium2 NeuronCores.

Math refactor of the reference:
    e = [x_i, x_j - x_i]; h = relu(e@W1 + b1)@W2 + b2; agg = segment_mean(h, dst)
    out = relu(relu(agg)@W3 + b3)@W4 + b4
is rewritten as
    u = x @ (W1a - W1b) + b1        (per node, W1a = W1[:128], W1b = W1[128:])
    v = x @ W1b                      (per node)
    m_i = mean_{e: dst=i} relu(u_i + v_{src(e)})
    agg_i = m_i @ W2 + b2            (for deg>0; isolated nodes fixed on host)
    out = relu(relu(agg) @ W3 + b3) @ W4 + b4

Device mapping (one SPMD program, 8 cores, no collectives):
 - nodes sorted by (deg_lo, deg_hi) and dealt round-robin into
   49 positions x 8 cores x 128 lanes; each core owns 6272 node slots.
 - bf16 throughout (inputs quantized on host; 2e-2 rel tol has plenty of
   headroom, ~5e-3 measured): v rows are 256B, DVE/ACT run in 2x mode.
 - v table in DRAM: rows 0..127 = NEG pad, 128..50175 = v[node],
   50176..50303 = NEG pad. Window split at node 17408 so only 136 of the
   391 v tiles must be built before the A-sweep gathers start; the other
   255 are built concurrently with the A sweep.
 - phase B sweeps the lo window (A slots) then the hi window (B slots)
   with GLOBAL 8-slot/1024-idx gather chunks spanning position
   boundaries (1024 idxs = HW limit per dma_gather). Per chunk: DVE adds
   u (free-dim broadcast) per position segment, ACT relu once, DVE
   halving tree-sum per segment into sA.
 - trailing MLP batched 4 positions at a time: ACT recip-scale, PE
   transpose to [f, node], 512-wide matmuls, output written transposed
   [F, OWN] and fixed up on host.
"""
import sys

sys.path.insert(0, "/opt/trn_rl_repo")

import numpy as np
import ml_dtypes

BF16 = ml_dtypes.bfloat16

N = 50000
F = 128
P = 128
N_CORES = 8
N_PAD = 50048            # 391 * 128, node count padded for 128-tiles
RANKN = 49 * 1024        # 50176 rank slots (49 positions x 8 cores x 128 lanes)
NPOS = 49
OWN = NPOS * P           # 6272 node slots per core
SPLIT = 17408            # src < SPLIT -> lo window, else hi window
V_ROWS = N_PAD + 2 * P   # 128 NEG + nodes + 128 NEG
LO_ROWS = SPLIT + P      # lo window rows [0, 17536)
PAD_LO = 0
PAD_HI = LO_ROWS + N_PAD - SPLIT - LO_ROWS + P * 0  # placeholder, set below
PAD_HI = V_ROWS - LO_ROWS - P  # 32640: first NEG row in hi-window coords
NEG = -1.0e30
CH = 8                   # slots per gather chunk (1024 idxs = HW limit)
SINGLE_PACKET = False

_compiled_cache = {}
PROFILE = False
LAST_EXEC_NS = None
LAST_TRACE = None
LAST_TMPDIR = None


def _preprocess(x, src, dst):
    """Host-side integer/index preprocessing. Returns per-core tensors and
    the static config that shapes the device program."""
    E = src.shape[0]
    A_mask = src < SPLIT

    cnt = np.bincount(dst, minlength=N_PAD).astype(np.int64)
    cntA = np.bincount(dst[A_mask], minlength=N_PAD).astype(np.int64)
    cntB = cnt - cntA

    # sort nodes by (deg_lo desc, deg_hi desc): per-position max degree ~ mean
    order = np.lexsort((-cntB, -cntA))           # [N_PAD] old ids by rank
    old_of_rank = np.full(RANKN, -1, dtype=np.int64)
    old_of_rank[:N_PAD] = order
    rank_of_old = np.empty(N_PAD, dtype=np.int64)
    rank_of_old[order] = np.arange(N_PAD)

    cntA_r = np.zeros(RANKN, dtype=np.int64)
    cntB_r = np.zeros(RANKN, dtype=np.int64)
    cntA_r[:N_PAD] = cntA[order]
    cntB_r[:N_PAD] = cntB[order]
    K_A = cntA_r.reshape(NPOS, 1024).max(axis=1)          # [49]
    K_B = cntB_r.reshape(NPOS, 1024).max(axis=1)
    baseA = np.concatenate([[0], np.cumsum(K_A)])          # [50]
    baseB = np.concatenate([[0], np.cumsum(K_B)])
    totA, totB = int(baseA[-1]), int(baseB[-1])

    # place each edge into its (core, half, flat slot)
    r_dst = rank_of_old[dst]
    half = (~A_mask).astype(np.int64)
    eorder = np.lexsort((src, half, r_dst))
    rs = r_dst[eorder]
    hs = half[eorder]
    ss = src[eorder]
    grp = rs * 2 + hs
    newg = np.r_[True, np.diff(grp) != 0]
    gid = np.cumsum(newg) - 1
    first = np.flatnonzero(newg)
    j = np.arange(E) - first[gid]                # slot index within (node, half)

    p = rs // 1024
    w = rs % 1024
    core = w // P
    lane = w % P

    idxA = [np.full(totA * P, PAD_LO, dtype=np.int16) for _ in range(N_CORES)]
    idxB = [np.full(totB * P, PAD_HI, dtype=np.int16) for _ in range(N_CORES)]
    flatA = baseA[p] * P + j * P + lane
    flatB = baseB[p] * P + j * P + lane
    valA = (ss + P).astype(np.int16)             # lo row = node + 128
    valB = (ss - SPLIT).astype(np.int16)         # hi-window row
    for c in range(N_CORES):
        mA = (core == c) & (hs == 0)
        mB = (core == c) & (hs == 1)
        idxA[c][flatA[mA]] = valA[mA]
        idxB[c][flatB[mB]] = valB[mB]

    def wrap(flat):
        # dma_gather idx layout: [16, n/16] with elem i at [i%16, i//16],
        # replicated across the 8 gpsimd cores -> [128, n/16]
        if flat.size == 0:
            return np.zeros((P, 1), dtype=np.int16)
        wr = flat.reshape(-1, 16).T.copy()
        return np.tile(wr, (8, 1))

    idxA_t = [wrap(a) for a in idxA]
    idxB_t = [wrap(b) for b in idxB]

    # transposed node features (bf16) + per-core own slices
    xT = np.zeros((F, N_PAD), dtype=BF16)
    xT[:, :N] = x.T.astype(BF16)
    recip_all = (1.0 / np.maximum(cnt, 1)).astype(np.float32)

    xT_own, recip_own, ranks_core = [], [], []
    for c in range(N_CORES):
        ranks = (np.arange(NPOS)[:, None] * 1024 + c * P + np.arange(P)[None, :])
        ranks = ranks.reshape(-1)                # [6272] rank per (p, lane)
        olds = old_of_rank[ranks]
        xo = np.zeros((F, OWN), dtype=BF16)
        valid = olds >= 0
        xo[:, valid] = xT[:, olds[valid]]
        xT_own.append(np.ascontiguousarray(xo))
        rc_flat = np.zeros(OWN, dtype=np.float32)
        rc_flat[valid] = recip_all[olds[valid]]
        # rc[lane, p] = recip of rank (p, lane)
        rc = np.ascontiguousarray(rc_flat.reshape(NPOS, P).T)
        recip_own.append(rc)
        ranks_core.append(olds)

    cfg = (tuple(int(k) for k in K_A), tuple(int(k) for k in K_B))
    return dict(
        xT=np.ascontiguousarray(xT), xT_own=xT_own, recip_own=recip_own,
        idxA=idxA_t, idxB=idxB_t, K_A=K_A, K_B=K_B,
        ranks_core=ranks_core, cnt=cnt, cfg=cfg,
    )


def _chunks(tot):
    return [(s0, min(CH, tot - s0)) for s0 in range(0, tot, CH)]


def _segments(base, s0, sw):
    """Positions overlapping chunk slot range [s0, s0+sw) -> (a, b, p) local."""
    segs = []
    for p in range(NPOS):
        a = max(int(base[p]), s0)
        b = min(int(base[p + 1]), s0 + sw)
        if a < b:
            segs.append((a - s0, b - s0, p))
    return segs


def _build_program(cfg):
    """Build + compile the SPMD bass program for the given (K_A, K_B)."""
    import concourse.bass as bass
    import concourse.bacc as bacc
    import concourse.mybir as mybir
    import concourse.tile as tile
    from concourse.masks import make_identity

    K_A, K_B = cfg
    f32 = mybir.dt.float32
    bf16 = mybir.dt.bfloat16
    i16 = mybir.dt.int16
    totA = sum(K_A)
    totB = sum(K_B)
    baseA = np.concatenate([[0], np.cumsum(K_A)]).astype(int)
    baseB = np.concatenate([[0], np.cumsum(K_B)]).astype(int)

    nc = bacc.Bacc("TRN2", target_bir_lowering=False, debug=False,
                   num_devices=N_CORES, num_swdge_queues=4)

    xT_d = nc.dram_tensor("xT", [F, N_PAD], bf16, kind="ExternalInput")
    xTo_d = nc.dram_tensor("xT_own", [F, OWN], bf16, kind="ExternalInput")
    Wd_d = nc.dram_tensor("Wd", [F, F], bf16, kind="ExternalInput")
    W1b_d = nc.dram_tensor("W1b", [F, F], bf16, kind="ExternalInput")
    W2_d = nc.dram_tensor("W2", [F, F], bf16, kind="ExternalInput")
    W3_d = nc.dram_tensor("W3", [F, 64], bf16, kind="ExternalInput")
    W4_d = nc.dram_tensor("W4", [64, F], bf16, kind="ExternalInput")
    b1b_d = nc.dram_tensor("b1_bcast", [P, F], f32, kind="ExternalInput")
    b2c_d = nc.dram_tensor("b2col", [F, 1], f32, kind="ExternalInput")
    b3c_d = nc.dram_tensor("b3col", [64, 1], f32, kind="ExternalInput")
    b4c_d = nc.dram_tensor("b4col", [F, 1], f32, kind="ExternalInput")
    recip_d = nc.dram_tensor("recip", [P, NPOS], f32, kind="ExternalInput")
    idxA_d = nc.dram_tensor("idxA", [P, max(totA * 8, 1)], i16,
                            kind="ExternalInput")
    idxB_d = nc.dram_tensor("idxB", [P, max(totB * 8, 1)], i16,
                            kind="ExternalInput")

    v_d = nc.dram_tensor("v", [V_ROWS, F], bf16)
    out_d = nc.dram_tensor("out", [F, OWN], f32, kind="ExternalOutput")

    chunksA = _chunks(totA)
    chunksB = _chunks(totB)

    with tile.TileContext(nc) as tc:
        with (
            tc.tile_pool(name="persist", bufs=1) as pers,
            tc.tile_pool(name="stage", bufs=4) as stage,
            tc.tile_pool(name="zpool", bufs=12) as zpool,
            tc.tile_pool(name="small", bufs=3) as small,
            tc.tile_pool(name="psV", bufs=2, space="PSUM") as psV,
            tc.tile_pool(name="psT", bufs=2, space="PSUM") as psT,
            tc.tile_pool(name="psG", bufs=2, space="PSUM") as psG,
        ):
            # ---- constants ----
            Wd_t = pers.tile([F, F], bf16)
            nc.sync.dma_start(out=Wd_t[:], in_=Wd_d[:])
            W1b_t = pers.tile([F, F], bf16)
            nc.sync.dma_start(out=W1b_t[:], in_=W1b_d[:])
            W2_t = pers.tile([F, F], bf16)
            nc.sync.dma_start(out=W2_t[:], in_=W2_d[:])
            W3_t = pers.tile([F, 64], bf16)
            nc.sync.dma_start(out=W3_t[:], in_=W3_d[:])
            W4_t = pers.tile([64, F], bf16)
            nc.sync.dma_start(out=W4_t[:], in_=W4_d[:])
            b1b_t = pers.tile([P, F], f32)
            nc.sync.dma_start(out=b1b_t[:], in_=b1b_d[:])
            b2c_t = pers.tile([F, 1], f32)
            nc.sync.dma_start(out=b2c_t[:], in_=b2c_d[:])
            b3c_t = pers.tile([64, 1], f32)
            nc.sync.dma_start(out=b3c_t[:], in_=b3c_d[:])
            b4c_t = pers.tile([F, 1], f32)
            nc.sync.dma_start(out=b4c_t[:], in_=b4c_d[:])
            recip_t = pers.tile([P, NPOS], f32)
            nc.sync.dma_start(out=recip_t[:], in_=recip_d[:])
            idxA_t = pers.tile([P, max(totA * 8, 1)], i16)
            nc.sync.dma_start(out=idxA_t[:], in_=idxA_d[:])
            idxB_t = pers.tile([P, max(totB * 8, 1)], i16)
            nc.sync.dma_start(out=idxB_t[:], in_=idxB_d[:])
            ident = pers.tile([P, P], bf16)
            make_identity(nc, ident[:])

            neg_t = pers.tile([P, F], bf16)
            nc.vector.memset(neg_t[:], NEG)
            nc.sync.dma_start(out=v_d[0:P, :], in_=neg_t[:])
            nc.sync.dma_start(out=v_d[V_ROWS - P:V_ROWS, :], in_=neg_t[:])

            u_t = pers.tile([P, OWN], bf16)     # u[lane, pos*F + f]
            sA_t = pers.tile([P, OWN], bf16)    # s[lane, pos*F + f]
            nc.vector.memset(sA_t[:], 0.0)

            # ---- u = x@Wd + b1 (lane-major: u_t block p = [node, f]) ----
            for p4 in range(0, NPOS, 4):
                pw = min(4, NPOS - p4)
                xo = stage.tile([F, 512], bf16, tag="xo")
                nc.sync.dma_start(out=xo[:, :pw * P],
                                  in_=xTo_d[:, p4 * P:(p4 + pw) * P])
                for i in range(pw):
                    up = psV.tile([P, F], f32, tag="pv")
                    nc.tensor.matmul(out=up[:], lhsT=xo[:, i * P:(i + 1) * P],
                                     rhs=Wd_t[:], start=True, stop=True)
                    nc.vector.tensor_tensor(
                        out=u_t[:, (p4 + i) * P:(p4 + i + 1) * P],
                        in0=up[:], in1=b1b_t[:], op=mybir.AluOpType.add)

            # ---- v tile builder (node tile t -> v rows 128+t*128) ----
            NT = N_PAD // P          # 391 node tiles
            _alt = [0]

            def emit_v_tiles(t0, t1):
                b0 = t0
                while b0 < t1:
                    bt = min(4, t1 - b0)
                    xs = stage.tile([F, 512], bf16, tag="xs")
                    nc.sync.dma_start(out=xs[:, :bt * P],
                                      in_=xT_d[:, b0 * P:(b0 + bt) * P])
                    vstage = stage.tile([P, 4, F], bf16, tag="vs")
                    for ti in range(bt):
                        pv = psV.tile([P, F], f32, tag="pv")
                        nc.tensor.matmul(out=pv[:],
                                         lhsT=xs[:, ti * P:(ti + 1) * P],
                                         rhs=W1b_t[:], start=True, stop=True)
                        if _alt[0] % 2 == 0:
                            nc.vector.tensor_copy(out=vstage[:, ti, :],
                                                  in_=pv[:])
                        else:
                            nc.scalar.activation(
                                vstage[:, ti, :], pv[:],
                                mybir.ActivationFunctionType.Copy)
                        _alt[0] += 1
                    r0 = P + b0 * P
                    nc.sync.dma_start(
                        out=v_d[r0:r0 + bt * P, :]
                            .rearrange("(t q) f -> q t f", q=P),
                        in_=vstage[:, :bt, :])
                    b0 += bt

            LO_T = SPLIT // P        # 136 node tiles in the lo window
            emit_v_tiles(0, LO_T)

            hi_next = [LO_T]

            def emit_hi_share(n_tiles):
                t1 = min(NT, hi_next[0] + n_tiles)
                if t1 > hi_next[0]:
                    emit_v_tiles(hi_next[0], t1)
                    hi_next[0] = t1

            _q = [0]

            def do_chunk(ci, s0, sw, base, idx_t, in_view):
                z = zpool.tile([P, CH, F], bf16, tag="z")
                nc.gpsimd.dma_gather(
                    out_ap=z[:, :sw, :], in_ap=in_view,
                    idxs_ap=idx_t[:, s0 * 8:(s0 + sw) * 8],
                    num_idxs=sw * P, num_idxs_reg=sw * P, elem_size=F,
                    single_packet=SINGLE_PACKET, queue_num=_q[0] % 4)
                _q[0] += 1
                segs = _segments(base, s0, sw)
                for (a, b, p) in segs:
                    u_b = (u_t[:, p * P:(p + 1) * P]
                           .rearrange("q (k f) -> q k f", k=1)
                           .to_broadcast([P, b - a, F]))
                    nc.vector.tensor_tensor(out=z[:, a:b, :], in0=z[:, a:b, :],
                                            in1=u_b, op=mybir.AluOpType.add)
                nc.scalar.activation(z[:, :sw, :], z[:, :sw, :],
                                     mybir.ActivationFunctionType.Relu)
                for (a, b, p) in segs:
                    w = b - a
                    while w > 1:
                        h = w // 2
                        nc.vector.tensor_tensor(
                            out=z[:, a:a + h, :], in0=z[:, a:a + h, :],
                            in1=z[:, a + w - h:a + w, :],
                            op=mybir.AluOpType.add)
                        w -= h
                    sl = sA_t[:, p * P:(p + 1) * P]
                    nc.vector.tensor_tensor(out=sl, in0=sl, in1=z[:, a, :],
                                            op=mybir.AluOpType.add)

            def emit_mlp_group(p0, pw):
                mst = small.tile([P, 4, F], bf16, tag="mst")
                for i in range(pw):
                    nc.scalar.activation(mst[:, i, :],
                                         sA_t[:, (p0 + i) * P:(p0 + i + 1) * P],
                                         mybir.ActivationFunctionType.Copy,
                                         scale=recip_t[:, p0 + i:p0 + i + 1])
                mT = small.tile([P, 4 * P], bf16, tag="mT")
                for i in range(pw):
                    pt = psT.tile([P, P], bf16, tag="pt")
                    nc.tensor.transpose(out=pt[:], in_=mst[:, i, :],
                                        identity=ident[:])
                    nc.vector.tensor_copy(out=mT[:, i * P:(i + 1) * P],
                                          in_=pt[:])
                o1 = psG.tile([P, 512], f32, tag="pg")
                nc.tensor.matmul(out=o1[:, :pw * P], lhsT=W2_t[:],
                                 rhs=mT[:, :pw * P], start=True, stop=True)
                h1 = small.tile([F, 512], bf16, tag="h1")
                nc.scalar.activation(h1[:, :pw * P], o1[:, :pw * P],
                                     mybir.ActivationFunctionType.Relu,
                                     bias=b2c_t[:, :1])
                o2 = psG.tile([64, 512], f32, tag="pg2")
                nc.tensor.matmul(out=o2[:, :pw * P], lhsT=W3_t[:],
                                 rhs=h1[:, :pw * P], start=True, stop=True)
                h2 = small.tile([64, 512], bf16, tag="h2")
                nc.scalar.activation(h2[:, :pw * P], o2[:, :pw * P],
                                     mybir.ActivationFunctionType.Relu,
                                     bias=b3c_t[:, :1])
                o3 = psG.tile([P, 512], f32, tag="pg")
                nc.tensor.matmul(out=o3[:, :pw * P], lhsT=W4_t[:],
                                 rhs=h2[:, :pw * P], start=True, stop=True)
                yT = small.tile([F, 512], f32, tag="yT")
                nc.scalar.activation(yT[:, :pw * P], o3[:, :pw * P],
                                     mybir.ActivationFunctionType.Identity,
                                     bias=b4c_t[:, :1])
                nc.sync.dma_start(out=out_d[:, p0 * P:(p0 + pw) * P],
                                  in_=yT[:, :pw * P])

            # ---- A sweep (lo window) with hi v build interleaved ----
            lo_view = v_d[:LO_ROWS, :]
            hi_share = (NT - LO_T) // max(len(chunksA), 1) + 1
            for ci, (s0, sw) in enumerate(chunksA):
                do_chunk(ci, s0, sw, baseA, idxA_t, lo_view)
                emit_hi_share(hi_share)
            emit_hi_share(NT)        # any leftover hi tiles

            # ---- B sweep (hi window) + batched MLP when groups complete ----
            hi_view = v_d[LO_ROWS:, :]
            groups = [(g0, min(4, NPOS - g0)) for g0 in range(0, NPOS, 4)]
            mlp_after = {}           # chunk idx -> [(p0, pw)]
            pre_b = []
            for (g0, gw) in groups:
                lastp = [p for p in range(g0, g0 + gw) if K_B[p] > 0]
                if not lastp:
                    pre_b.append((g0, gw))
                else:
                    ci = (int(baseB[lastp[-1] + 1]) - 1) // CH
                    mlp_after.setdefault(ci, []).append((g0, gw))
            for (g0, gw) in pre_b:
                emit_mlp_group(g0, gw)
            for ci, (s0, sw) in enumerate(chunksB):
                do_chunk(ci, s0, sw, baseB, idxB_t, hi_view)
                for (g0, gw) in mlp_after.get(ci, []):
                    emit_mlp_group(g0, gw)

    nc.compile()
    return nc


def _install_ntff_hook():
    """The agent image's antenv lacks axon_hooks; synthesize it so
    run_bass_kernel_spmd(trace=True) can NTFF-profile via libaxon."""
    import types
    if "antenv.axon_hooks" in sys.modules:
        return
    try:
        mod = types.ModuleType("antenv.axon_hooks")
        holder = {}
        mod.set_axon_ntff_profile_hook = lambda h: holder.__setitem__("h", h)
        mod.get_axon_ntff_profile_hook = lambda: holder.get("h")
        import antenv
        sys.modules["antenv.axon_hooks"] = mod
        antenv.axon_hooks = mod
        from trn_agent_boot.trn_boot import _ntff_profile_via_ctypes
        mod.set_axon_ntff_profile_hook(
            _ntff_profile_via_ctypes("/opt/axon/libaxon_pjrt.so"))
    except Exception:
        pass


def kernel(x, edge_index, W1, b1, W2, b2, W3, b3, W4, b4):
    from concourse.bass_utils import run_bass_kernel_spmd

    x = np.asarray(x, dtype=np.float32)
    edge_index = np.asarray(edge_index)
    src = np.asarray(edge_index[0], dtype=np.int64)
    dst = np.asarray(edge_index[1], dtype=np.int64)
    W1 = np.asarray(W1, dtype=np.float32)
    b1 = np.asarray(b1, dtype=np.float32)
    W2 = np.asarray(W2, dtype=np.float32)
    b2 = np.asarray(b2, dtype=np.float32)
    W3 = np.asarray(W3, dtype=np.float32)
    b3 = np.asarray(b3, dtype=np.float32)
    W4 = np.asarray(W4, dtype=np.float32)
    b4 = np.asarray(b4, dtype=np.float32)

    pre = _preprocess(x, src, dst)
    cfg = pre["cfg"]
    if cfg not in _compiled_cache:
        _compiled_cache.clear()
        _compiled_cache[cfg] = _build_program(cfg)
    nc = _compiled_cache[cfg]

    W1a = W1[:F]
    W1b = W1[F:]
    Wd = np.ascontiguousarray((W1a - W1b).astype(BF16))
    b1_bcast = np.tile(b1[None, :], (P, 1)).astype(np.float32)

    in_maps = []
    for c in range(N_CORES):
        in_maps.append({
            "xT": pre["xT"], "xT_own": pre["xT_own"][c],
            "Wd": Wd, "W1b": np.ascontiguousarray(W1b.astype(BF16)),
            "W2": W2.astype(BF16), "W3": W3.astype(BF16),
            "W4": W4.astype(BF16),
            "b1_bcast": b1_bcast,
            "b2col": np.ascontiguousarray(b2[:, None]),
            "b3col": np.ascontiguousarray(b3[:, None]),
            "b4col": np.ascontiguousarray(b4[:, None]),
            "recip": pre["recip_own"][c],
            "idxA": pre["idxA"][c],
            "idxB": pre["idxB"][c],
        })

    kw = {}
    if PROFILE:
        import tempfile
        _install_ntff_hook()
        kw = dict(trace=True, tmpdir=tempfile.mkdtemp(prefix="gnn_trace_"))
    last_err = None
    for attempt in range(3):
        try:
            res = run_bass_kernel_spmd(nc, in_maps, list(range(N_CORES)), **kw)
            break
        except Exception as e:  # device flake: retry
            last_err = e
            import time
            time.sleep(15)
    else:
        raise last_err

    global LAST_EXEC_NS, LAST_TRACE, LAST_TMPDIR
    LAST_EXEC_NS = res.exec_time_ns
    LAST_TRACE = (res.instructions_and_trace[1]
                  if res.instructions_and_trace else None)
    LAST_TMPDIR = kw.get("tmpdir")

    out = np.zeros((N, F), dtype=np.float32)
    for c in range(N_CORES):
        olds = pre["ranks_core"][c]
        valid = (olds >= 0) & (olds < N)
        out[olds[valid]] = res.results[c]["out"].T[valid]

    # isolated nodes: reference gives agg=0 -> out = relu(b3)@W4 + b4
    iso = np.flatnonzero(pre["cnt"][:N] == 0)
    if iso.size:
        row = np.maximum(b3, 0.0) @ W4 + b4
        out[iso] = row[None, :]
    return out


if __name__ == "__main__":
    sys.path.insert(0, "/root/problem")
    import jax
    import reference
    with jax.default_device(jax.devices("cpu")[0]):
        inputs = reference.setup_inputs()
        expected = np.asarray(reference.reference(**inputs))
    inputs = {k: np.asarray(v) for k, v in inputs.items()}
    got = kernel(**inputs)
    denom = np.abs(expected).max()
    rel = np.abs(got - expected).max() / denom
    print("Relative error:", rel)
    print("PASS" if rel < 2e-2 else "FAIL")
